# revision 10
# baseline (speedup 1.0000x reference)
"""Trainium2 Bass kernel for nn_Bases (gnn_message_passing), 8-core SPMD.

Sharding: triplets sorted by trip_out and split 125k/core (graph
partitioning); the induced contiguous out-edge range (~25k edges) defines
each core's edge shard.  h rows needed per shard are host-gathered
(distribution-level all-gather per the problem's sharding hint); all model
math runs on device.
"""

import sys

sys.path.insert(0, "/opt/trn_rl_repo")

import numpy as np

import concourse.bass as bass
import concourse.mybir as mybir
import concourse.tile as tile
from concourse import bacc
from concourse.bass_utils import run_bass_kernel_spmd
from concourse.masks import make_identity

# ---- problem constants ----
NCORES = 8
NA, NE, NT = 8000, 200000, 1000000
R = 128          # NUM_RADIAL
S = 7            # NUM_SPH
C = 16           # EMB_CBF
EA = 256         # EMB_ATOM
EO = 512         # EMB_EDGE
CUT = 12.0
PENV = 5
COEFF = -0.5 * 127.0 * 127.0      # -0.5/(offsets[1]-offsets[0])**2
ENV_A = -(PENV + 1) * (PENV + 2) / 2.0
ENV_B = PENV * (PENV + 2.0)
ENV_C = -PENV * (PENV + 1) / 2.0

# ---- sharding constants ----
TSH = NT // NCORES                # 125000 triplets per core
IPP = 1024                        # triplet columns per partition
TPAD = 128 * IPP                  # 131072 padded triplets per core
NTC = 64                          # triplet columns per chunk
NCHUNK = IPP // NTC               # 16
GIDX = 128 * NTC                  # 8192 gathered rows per chunk
GSUB = 2048                       # rows per dma_gather call
NEP = 26624                       # padded edge range per core (208 tiles)
TE = NEP // 128

f32 = mybir.dt.float32
f32r = mybir.dt.float32r
i16 = mybir.dt.int16
ALU = mybir.AluOpType
AFT = mybir.ActivationFunctionType

_NC = None
SIM_SAFE = False
PHASES = "ET"   # which phases to emit (debug bisect)


def _build():
    nc = bacc.Bacc(None, target_bir_lowering=False)

    dist_pt = nc.declare_dram_parameter("dist_pt", [128, TE], f32, isOutput=False)
    vedge = nc.declare_dram_parameter("vedge", [128, TE, 3], f32, isOutput=False)
    hsT = nc.declare_dram_parameter("hsT", [EA, NEP], f32r, isOutput=False)
    htT = nc.declare_dram_parameter("htT", [EA, NEP], f32r, isOutput=False)
    Wp = nc.declare_dram_parameter("Wp", [128, 128], f32, isOutput=False)
    W16 = nc.declare_dram_parameter("W16", [128, 48], f32, isOutput=False)
    We = nc.declare_dram_parameter("We", [640, EO], f32r, isOutput=False)
    offs = nc.declare_dram_parameter("offs", [128, 128], f32, isOutput=False)
    idxg = nc.declare_dram_parameter("idxg", [NCHUNK, 128, GIDX // 16], i16, isOutput=False)
    vin = nc.declare_dram_parameter("vin", [128, IPP, 3], f32, isOutput=False)

    m_out = nc.declare_dram_parameter("m_out", [NEP, EO], f32, isOutput=True)
    r3_out = nc.declare_dram_parameter("r3_out", [NEP, 48], f32, isOutput=True)
    ce_out = nc.declare_dram_parameter("ce_out", [128, IPP, C], f32, isOutput=True)

    G_d = nc.dram_tensor("G_d", [NEP, 128], f32)

    hsT_r = hsT.rearrange("(j p) e -> p j e", p=128)
    htT_r = htT.rearrange("(j p) e -> p j e", p=128)
    We_r = We.rearrange("(k p) n -> p k n", p=128)

    with tile.TileContext(nc) as tc:
        with tc.tile_pool(name="const", bufs=1) as const:
            ident = const.tile([128, 128], f32)
            make_identity(nc, ident[:])
            offs_sb = const.tile([128, 128], f32)
            nc.sync.dma_start(out=offs_sb[:], in_=offs[:])
            Wp_sb = const.tile([128, 128], f32)
            nc.sync.dma_start(out=Wp_sb[:], in_=Wp[:])
            W16_sb = const.tile([128, 48], f32)
            nc.sync.dma_start(out=W16_sb[:], in_=W16[:])
            We_sb = const.tile([128, 5, EO], f32r)
            nc.sync.dma_start(out=We_sb[:], in_=We_r[:])
            dist_sb = const.tile([128, TE], f32)
            nc.sync.dma_start(out=dist_sb[:], in_=dist_pt[:])
            ve_sb = const.tile([128, TE, 3], f32)
            nc.sync.dma_start(out=ve_sb[:], in_=vedge[:])

            # ds = dist/CUT ; env = 1 + ds^5*(A + B*ds + C*ds^2)
            ds_sb = const.tile([128, TE], f32)
            nc.vector.tensor_scalar_mul(ds_sb[:], dist_sb[:], 1.0 / CUT)
            ds2 = const.tile([128, TE], f32)
            nc.vector.tensor_mul(ds2[:], ds_sb[:], ds_sb[:])
            ds5 = const.tile([128, TE], f32)
            nc.vector.tensor_mul(ds5[:], ds2[:], ds2[:])          # ds^4
            nc.vector.tensor_mul(ds5[:], ds5[:], ds_sb[:])        # ds^5
            env_sb = const.tile([128, TE], f32)
            nc.vector.tensor_scalar_mul(env_sb[:], ds2[:], ENV_C)  # C*ds^2
            nc.vector.scalar_tensor_tensor(
                out=env_sb[:], in0=ds_sb[:], scalar=ENV_B, in1=env_sb[:],
                op0=ALU.mult, op1=ALU.add,
            )                                                      # B*ds + C*ds^2
            nc.vector.tensor_scalar_add(env_sb[:], env_sb[:], ENV_A)
            nc.vector.tensor_mul(env_sb[:], env_sb[:], ds5[:])
            nc.vector.tensor_scalar_add(env_sb[:], env_sb[:], 1.0)

            # ---------------- Phase E: per-edge-tile pipeline ----------------
            with (
                tc.tile_pool(name="ep", bufs=3) as ep,
                tc.tile_pool(name="eps", bufs=2, space="PSUM") as eps,
            ):
                for t in range(TE if "E" in PHASES else 0):
                    e0 = t * 128
                    t0 = ep.tile([128, 128], f32, tag="t0")
                    nc.vector.tensor_scalar(
                        out=t0[:], in0=offs_sb[:], scalar1=ds_sb[:, t : t + 1],
                        scalar2=None, op0=ALU.subtract,
                    )
                    sq = ep.tile([128, 128], f32, tag="sq")
                    nc.scalar.square(sq[:], t0[:])
                    rex = ep.tile([128, 128], f32, tag="rex")
                    nc.scalar.activation(rex[:], sq[:], AFT.Exp, scale=COEFF)
                    rad = ep.tile([128, 128], f32, tag="rad")
                    nc.vector.tensor_scalar(
                        out=rad[:], in0=rex[:], scalar1=env_sb[:, t : t + 1],
                        scalar2=None, op0=ALU.mult,
                    )
                    radT_ps = eps.tile([128, 128], f32, tag="radT")
                    nc.tensor.transpose(radT_ps[:], rad[:], ident[:])
                    radT = ep.tile([128, 128], f32r, tag="radTs")
                    nc.vector.tensor_copy(out=radT[:], in_=radT_ps[:])
                    radT32 = ep.tile([128, 128], f32, tag="radT32")
                    nc.vector.tensor_copy(out=radT32[:], in_=radT_ps[:])

                    # G table tile (monomial cbf basis + v in cols 112:115)
                    g_ps = eps.tile([128, 128], f32, tag="g")
                    nc.tensor.matmul(g_ps[:], lhsT=radT32[:], rhs=Wp_sb[:], start=True, stop=True)
                    g_sb = ep.tile([128, 128], f32, tag="gs")
                    nc.scalar.copy(g_sb[:], g_ps[:])
                    nc.vector.tensor_copy(out=g_sb[:, 112:115], in_=ve_sb[:, t, :])
                    nc.sync.dma_start(out=G_d[e0 : e0 + 128, :], in_=g_sb[:])

                    # the three 16-wide heads
                    r3_ps = eps.tile([128, 48], f32, tag="r3")
                    nc.tensor.matmul(r3_ps[:], lhsT=radT32[:], rhs=W16_sb[:], start=True, stop=True)
                    r3_sb = ep.tile([128, 48], f32, tag="r3s")
                    nc.vector.tensor_copy(out=r3_sb[:], in_=r3_ps[:])
                    nc.sync.dma_start(out=r3_out[e0 : e0 + 128, :], in_=r3_sb[:])

                    # edge embedding: m = silu(m_in @ We) / 0.6  (We pre-divided by 0.6)
                    hs = ep.tile([128, 2, 128], f32r, tag="hs")
                    nc.sync.dma_start(out=hs[:], in_=hsT_r[:, :, e0 : e0 + 128])
                    ht = ep.tile([128, 2, 128], f32r, tag="ht")
                    nc.sync.dma_start(out=ht[:], in_=htT_r[:, :, e0 : e0 + 128])
                    m_ps = eps.tile([128, EO], f32, tag="m")
                    nc.tensor.matmul(m_ps[:], lhsT=hs[:, 0, :], rhs=We_sb[:, 0, :], start=True, stop=False)
                    nc.tensor.matmul(m_ps[:], lhsT=hs[:, 1, :], rhs=We_sb[:, 1, :], start=False, stop=False)
                    nc.tensor.matmul(m_ps[:], lhsT=ht[:, 0, :], rhs=We_sb[:, 2, :], start=False, stop=False)
                    nc.tensor.matmul(m_ps[:], lhsT=ht[:, 1, :], rhs=We_sb[:, 3, :], start=False, stop=False)
                    nc.tensor.matmul(m_ps[:], lhsT=radT[:], rhs=We_sb[:, 4, :], start=False, stop=True)
                    ms = ep.tile([128, EO], f32, tag="ms")
                    if SIM_SAFE:
                        # silu(x)/0.6 = x' * sigmoid(0.6*x') with x' = x/0.6
                        nc.scalar.activation(ms[:], m_ps[:], AFT.Sigmoid, scale=0.6)
                        nc.vector.tensor_mul(ms[:], ms[:], m_ps[:])
                    else:
                        nc.scalar.activation(ms[:], m_ps[:], AFT.Silu, scale=0.6)
                        nc.vector.tensor_scalar_mul(ms[:], ms[:], 1.0 / 0.6)
                    nc.sync.dma_start(out=m_out[e0 : e0 + 128, :], in_=ms[:])

            if PHASES == "ET":
                tc.strict_bb_all_engine_barrier()

            # ---------------- Phase T: per-triplet-chunk pipeline ----------------
            with tc.tile_pool(name="tp", bufs=2) as tp:
                for ci in range(NCHUNK if "T" in PHASES else 0):
                    i0 = ci * NTC
                    ix = tp.tile([128, GIDX // 16], i16, tag="ix")
                    nc.sync.dma_start(out=ix[:], in_=idxg[ci])
                    gt = tp.tile([128, NTC, 128], f32, tag="gt")
                    if "2" in PHASES:
                        # bisect: plain load instead of gather
                        nc.sync.dma_start(
                            out=gt[:],
                            in_=G_d.rearrange("(a n) c -> a n c", n=NTC)[
                                ci % (NEP // (128 * NTC)) * 128 : ci % (NEP // (128 * NTC)) * 128 + 128
                            ],
                        )
                    else:
                        nc.gpsimd.dma_gather(
                            out_ap=gt[:], in_ap=G_d[:], idxs_ap=ix[:],
                            num_idxs=GIDX, num_idxs_reg=GIDX, elem_size=128,
                            single_packet=False,
                        )
                    vi = tp.tile([128, NTC, 3], f32, tag="vi")
                    nc.sync.dma_start(out=vi[:], in_=vin[:, i0 : i0 + NTC, :])
                    if "1" in PHASES:
                        # bisect: skip DVE math, just move a gt slice out
                        ce1 = tp.tile([128, NTC, C], f32, tag="ce")
                        nc.vector.tensor_copy(out=ce1[:], in_=gt[:, :, 0:C])
                        nc.sync.dma_start(out=ce_out[:, i0 : i0 + NTC, :], in_=ce1[:])
                        continue

                    # cos_phi = clip(<v_out, v_in>) into xp slot 1
                    pr3 = tp.tile([128, NTC, 3], f32, tag="pr3")
                    nc.vector.tensor_mul(pr3[:], gt[:, :, 112:115], vi[:])
                    xp = tp.tile([128, NTC, 8], f32, tag="xp")
                    nc.vector.tensor_reduce(
                        out=xp[:, :, 1:2], in_=pr3[:], axis=mybir.AxisListType.X, op=ALU.add,
                    )
                    nc.vector.tensor_scalar(
                        out=xp[:, :, 1:2], in0=xp[:, :, 1:2],
                        scalar1=-1.0, scalar2=1.0, op0=ALU.max, op1=ALU.min,
                    )
                    nc.vector.memset(xp[:, :, 0:1], 1.0)
                    x1, x2, x3 = xp[:, :, 1:2], xp[:, :, 2:3], xp[:, :, 3:4]
                    x4, x5, x6 = xp[:, :, 4:5], xp[:, :, 5:6], xp[:, :, 6:7]
                    nc.vector.tensor_mul(x2, x1, x1)
                    nc.vector.tensor_mul(x3, x2, x1)
                    nc.vector.tensor_mul(x4, x2, x2)
                    nc.vector.tensor_mul(x5, x3, x2)
                    nc.vector.tensor_mul(x6, x3, x3)

                    # cir_e2e[c] = sum_k x^k * G[k*16+c]
                    g4 = gt[:, :, 0:112].rearrange("p n (k c) -> p n k c", k=S)
                    xb7 = xp[:, :, 0:7]
                    xb = bass.AP(tensor=xb7.tensor, offset=xb7.offset, ap=[*xb7.ap, [0, C]])
                    nc.vector.tensor_tensor(out=g4, in0=g4, in1=xb, op=ALU.mult)
                    gv = gt[:, :, 0:112].rearrange("p n (k c) -> p n c k", k=S)
                    ce = tp.tile([128, NTC, C], f32, tag="ce")
                    nc.vector.tensor_reduce(
                        out=ce[:], in_=gv, axis=mybir.AxisListType.X, op=ALU.add,
                    )
                    nc.sync.dma_start(out=ce_out[:, i0 : i0 + NTC, :], in_=ce[:])

    nc.compile()
    return nc


def _get_nc():
    global _NC
    if _NC is None:
        _NC = _build()
    return _NC


def _legendre_monomial_matrix():
    """A[s, k]: P_s(x)*pref_s = sum_k A[s,k] x^k  (float64)."""
    coefs = [np.array([1.0]), np.array([0.0, 1.0])]
    for l in range(2, S):
        prev, prev2 = coefs[-1], coefs[-2]
        c = np.zeros(l + 1)
        c[1:] += (2 * l - 1) * prev
        c[: l - 1] -= (l - 1) * prev2
        coefs.append(c / l)
    A = np.zeros((S, S))
    for s in range(S):
        A[s, : s + 1] = coefs[s] * np.sqrt((2 * s + 1) / (4 * np.pi))
    return A


def kernel(h, distance, vector, edge_idx_s, edge_idx_t, trip_in, trip_out,
           W_rbf_tint, W_cbf_tint, W_rbf_h, W_rbf_out, W_edge):
    h = np.asarray(h, dtype=np.float32)
    distance = np.asarray(distance, dtype=np.float32)
    vector = np.asarray(vector, dtype=np.float32)
    eis = np.asarray(edge_idx_s).astype(np.int64)
    eit = np.asarray(edge_idx_t).astype(np.int64)
    tin = np.asarray(trip_in).astype(np.int64)
    tout = np.asarray(trip_out).astype(np.int64)
    W_rbf_tint = np.asarray(W_rbf_tint, dtype=np.float32)
    W_cbf_tint = np.asarray(W_cbf_tint, dtype=np.float32)
    W_rbf_h = np.asarray(W_rbf_h, dtype=np.float32)
    W_rbf_out = np.asarray(W_rbf_out, dtype=np.float32)
    W_edge = np.asarray(W_edge, dtype=np.float32)

    # ---- graph partitioning: sort triplets by out-edge ----
    perm = np.argsort(tout, kind="stable")
    ts_out = tout[perm]
    ts_in = tin[perm]
    bounds = [0] + [int(ts_out[k * TSH]) for k in range(1, NCORES)] + [NE]

    # ---- shared weights ----
    A = _legendre_monomial_matrix()
    Wc = W_cbf_tint.astype(np.float64).reshape(R, S, C)
    Wp_ = np.einsum("rsc,sk->rkc", Wc, A).reshape(R, S * C)
    Wp_full = np.zeros((R, 128), dtype=np.float32)
    Wp_full[:, : S * C] = Wp_.astype(np.float32)
    W16_ = np.concatenate([W_rbf_tint, W_rbf_h, W_rbf_out], axis=1)
    WeS = np.ascontiguousarray(W_edge / 0.6)
    offs_rep = np.tile(np.linspace(0.0, 1.0, 128, dtype=np.float32)[None, :], (128, 1))

    in_maps = []
    for k in range(NCORES):
        ebase = bounds[k]
        gmax = int(ts_out[(k + 1) * TSH - 1])
        assert gmax - ebase + 1 <= NEP, (k, ebase, gmax)
        assert bounds[k + 1] - ebase <= NEP

        ge = np.arange(ebase, ebase + NEP)
        valid = ge < NE
        gec = np.where(valid, ge, 0)

        d = np.where(valid, distance[gec], 2.0 * CUT).astype(np.float32)
        dist_pt = np.ascontiguousarray(d.reshape(TE, 128).T)
        ve = np.where(valid[:, None], vector[gec], 0.0).astype(np.float32)
        vedge = np.ascontiguousarray(ve.reshape(TE, 128, 3).transpose(1, 0, 2))
        hsT = np.ascontiguousarray(h[eis[gec]].T)
        htT = np.ascontiguousarray(h[eit[gec]].T)

        loc = np.zeros(TPAD, dtype=np.int16)
        loc[:TSH] = (ts_out[k * TSH : (k + 1) * TSH] - ebase).astype(np.int16)
        idxg = np.empty((NCHUNK, 128, GIDX // 16), dtype=np.int16)
        p_ar = np.arange(128)
        il_ar = np.arange(NTC)
        for ci in range(NCHUNK):
            J = (p_ar[None, :] * IPP + ci * NTC + il_ar[:, None]).ravel()
            flat = loc[J]
            tile16 = flat.reshape(GIDX // 16, 16).T
            idxg[ci] = np.tile(tile16, (8, 1))

        tp_idx = np.zeros(TPAD, dtype=np.int64)
        tp_idx[:TSH] = ts_in[k * TSH : (k + 1) * TSH]
        vin = np.ascontiguousarray(vector[tp_idx].reshape(128, IPP, 3))

        in_maps.append({
            "dist_pt": dist_pt, "vedge": vedge, "hsT": hsT, "htT": htT,
            "Wp": Wp_full, "W16": W16_, "We": WeS, "offs": offs_rep,
            "idxg": idxg, "vin": vin,
        })

    nc = _get_nc()
    res = run_bass_kernel_spmd(nc, in_maps, list(range(NCORES)))

    m_full = np.empty((NE, EO), dtype=np.float32)
    rad_e2e = np.empty((NE, C), dtype=np.float32)
    atom_update = np.empty((NE, C), dtype=np.float32)
    output = np.empty((NE, C), dtype=np.float32)
    cir_e2e = np.empty((NT, C), dtype=np.float32)
    for k in range(NCORES):
        rk = res.results[k]
        n = bounds[k + 1] - bounds[k]
        sl = slice(bounds[k], bounds[k + 1])
        m_full[sl] = rk["m_out"][:n]
        r3 = rk["r3_out"][:n]
        rad_e2e[sl] = r3[:, 0:16]
        atom_update[sl] = r3[:, 16:32]
        output[sl] = r3[:, 32:48]
        ce = rk["ce_out"].reshape(TPAD, C)[:TSH]
        cir_e2e[perm[k * TSH : (k + 1) * TSH]] = ce

    return m_full, atom_update, output, rad_e2e, cir_e2e
